# revision 18
# baseline (speedup 1.0000x reference)
"""Trainium2 Bass kernel for the masked per-element grouped kernel problem.

Computes  y = Alpha @ [ (X_ref @ desc.T)**expK  masked to Z_ref[i]==Z[j] ]

Strategy (moe_routing): queries (N_in axis) and reference atoms only interact
when they share an element id (N_ELEM=4). Host-side routing gathers rows by
element; core c handles element e=c//2, query-half h=c%2. Each core runs a
dense pipeline on its element's atoms only:
    K  = X_e @ D_q.T          (contraction 512, bf16 matmuls, fp32 PSUM accum)
    K2 = K**2                 (ScalarE activation Square, PSUM->SBUF, bf16)
    y  = Alpha_e @ K2         (contraction over ref atoms, fp32 PSUM accum)
This is a 4x FLOP reduction over the dense masked formulation, on top of the
8-way data parallelism. A short dummy-matmul warmup flips the PE HAM clock
gate to full rate while the DMA head streams in. A few queries past the last
q-tile boundary (<=64 per element) are computed on host to avoid paying a
whole extra device q-tile for them.
"""

import math

import numpy as np

N_REF, N_IN, D_FEAT, P, N_ELEM = 4096, 8192, 512, 256, 4
N_CORES = 8
NF = D_FEAT // 128  # feature-dim contraction chunks

_COMPILED_CACHE: dict = {}


MAX_HOST_OVERFLOW = 64  # queries/element computed on host when it shaves a q-tile


def _pick_tiling(max_q: int) -> tuple[int, int]:
    """Pick (n_qtiles, qtile_width). Width must be in [256, 512] for full-rate
    matmuls and one PSUM bank; multiple of 32 for clean APs/DMA."""
    max_q = max(max_q, 256)
    n = max(1, math.ceil(max_q / 512))
    qw = math.ceil(max_q / n / 32) * 32
    qw = max(qw, 256)
    return n, qw


def _build_module(NR: int, NQT: int, QW: int, REP: int = 1, use_bf16: bool = False):
    """Build + compile the Bass module for padded shapes.

    Per-core DRAM inputs (blocked layouts produced on host):
      XT [NR, 128, 512]  block r: XT[r, p, f*128+j] = Xe[r*128+j, f*128+p]
      DT [NQT, 128, 4*QW] block qt: DT[qt, p, f*QW+q] = Dq[qt*QW+q, f*128+p]
      AT [128, NR*256]   AT[u, r*256+m] = Ae[r*128+u, m]  (Ae = Alpha[:, rix].T)
    Output:
      Y  [256, NQT*QW]
    """
    import concourse.tile as tile
    from concourse import bacc, mybir

    Q_pad = NQT * QW
    f32 = mybir.dt.float32
    ind = mybir.dt.bfloat16 if use_bf16 else mybir.dt.float32r

    nc = bacc.Bacc(
        "TRN2",
        target_bir_lowering=False,
        debug=False,
        enable_asserts=False,
        num_devices=N_CORES,
    )
    XT = nc.dram_tensor("XT", [NR, 128, NF * 128], ind, kind="ExternalInput")
    DT = nc.dram_tensor("DT", [NQT, 128, NF * QW], ind, kind="ExternalInput")
    AT = nc.dram_tensor("AT", [128, NR * 256], ind, kind="ExternalInput")
    Y = nc.dram_tensor("Y", [P, Q_pad], f32, kind="ExternalOutput")

    with tile.TileContext(nc) as tc:
        with (
            tc.tile_pool(name="xt", bufs=1) as xt_pool,
            tc.tile_pool(name="dt", bufs=1) as dt_pool,
            tc.tile_pool(name="at", bufs=1) as at_pool,
            tc.tile_pool(name="k2", bufs=NR + 2) as k2_pool,
            tc.tile_pool(name="ysb", bufs=1) as y_pool,
            tc.tile_pool(name="kps", bufs=4, space="PSUM") as kps_pool,
            tc.tile_pool(name="yps", bufs=3, space="PSUM") as yps_pool,
        ):
            # Persistent SBUF residents. Distinct tags -> own slots.
            dt_sb = [
                dt_pool.tile([128, NF * QW], ind, tag=f"dt{qt}", name=f"dt_sb{qt}")
                for qt in range(NQT)
            ]
            xt_sb = [
                xt_pool.tile([128, NF * 128], ind, tag=f"xt{r}", name=f"xt_sb{r}")
                for r in range(NR)
            ]
            at_sb = at_pool.tile([128, NR * 256], ind, tag="at", name="at_sb")

            for _rep in range(REP):
                _emit_body(nc, tc, mybir, NR, NQT, QW, XT, DT, AT, Y,
                           dt_sb, xt_sb, at_sb, k2_pool, y_pool, kps_pool, yps_pool,
                           use_bf16)

    nc.compile()
    return nc


def _emit_body(nc, tc, mybir, NR, NQT, QW, XT, DT, AT, Y,
               dt_sb, xt_sb, at_sb, k2_pool, y_pool, kps_pool, yps_pool,
               use_bf16=False):
    f32 = mybir.dt.float32
    ind = mybir.dt.bfloat16 if use_bf16 else mybir.dt.float32r
    SQUARE = mybir.ActivationFunctionType.Square
    P = 256

    # Loads. dt0 + xt* on the sync ring (first q-tile deps first); at and the
    # later q-tiles' desc blocks on the scalar HWDGE ring.
    half = NF * QW // 2
    nc.sync.dma_start(dt_sb[0][:, :half], DT[0][:, :half])
    nc.sync.dma_start(xt_sb[0][:], XT[0])
    nc.sync.dma_start(dt_sb[0][:, half:], DT[0][:, half:])
    for r in range(1, NR):
        nc.sync.dma_start(xt_sb[r][:], XT[r])
    nc.scalar.dma_start(at_sb[:], AT[:])
    for qt in range(1, NQT):
        nc.scalar.dma_start(dt_sb[qt][:], DT[qt])

    y_big = [
        y_pool.tile([128, NQT * QW], f32, tag=f"yb{p}", name=f"y_big{p}")
        for p in range(P // 128)
    ]
    # PE warmup: dummy matmuls against a memset tile (no DMA dependency) keep
    # the PE busy from ~1us through the DMA head, flipping the HAM clock gate
    # to 8/8 and holding it there so every real matmul runs at 2.4 GHz.
    w_ps = kps_pool.tile([128, 512], f32, tag="wps", name="w_ps", bufs=1)
    w_src = y_pool.tile([128, 512], ind, tag="wsrc", name="w_src", bufs=1)
    nc.gpsimd.memset(w_src[:], 0.0)
    for _w in range(15):
        nc.tensor.matmul(
            w_ps[:], w_src[:, 0:128], w_src[:], start=True, stop=True
        )
    for qt in range(NQT):
        k2_tiles = []
        for r in range(NR):
            k_ps = kps_pool.tile([128, QW], f32, tag="kps", name="k_ps")
            for f in range(NF):
                nc.tensor.matmul(
                    k_ps[:],
                    xt_sb[r][:, f * 128 : (f + 1) * 128],
                    dt_sb[qt][:, f * QW : (f + 1) * QW],
                    start=(f == 0),
                    stop=(f == NF - 1),
                )
            k2 = k2_pool.tile([128, QW], ind, tag="k2", name="k2")
            nc.scalar.activation(k2[:], k_ps[:], SQUARE)
            k2_tiles.append(k2)
        for p in range(P // 128):
            y_ps = yps_pool.tile([128, QW], f32, tag="yps", name="y_ps")
            for r in range(NR):
                nc.tensor.matmul(
                    y_ps[:],
                    at_sb[:, r * 256 + p * 128 : r * 256 + (p + 1) * 128],
                    k2_tiles[r][:],
                    start=(r == 0),
                    stop=(r == NR - 1),
                )
            nc.vector.tensor_copy(y_big[p][:, qt * QW : (qt + 1) * QW], y_ps[:])
            nc.sync.dma_start(
                Y[p * 128 : (p + 1) * 128, qt * QW : (qt + 1) * QW],
                y_big[p][:, qt * QW : (qt + 1) * QW],
            )


USE_BF16 = True


def _get_module(NR: int, NQT: int, QW: int):
    key = (NR, NQT, QW, USE_BF16)
    if key not in _COMPILED_CACHE:
        _COMPILED_CACHE[key] = _build_module(NR, NQT, QW, use_bf16=USE_BF16)
    return _COMPILED_CACHE[key]


def _host_fallback(Alpha, X_ref, desc, Z_ref, Z, expK):
    K = np.power(X_ref @ desc.T, expK)
    K *= Z_ref[:, None] == Z[None, :]
    return (Alpha @ K).astype(np.float32)


def _prepare_core_inputs(Alpha, X_ref, desc, Z_ref, Z):
    """Returns (in_maps, per_core_qidx, NR, NQT, QW)."""
    r_idx = [np.nonzero(Z_ref == e)[0] for e in range(N_ELEM)]
    q_idx = [np.nonzero(Z == e)[0] for e in range(N_ELEM)]

    # Split each element's queries across its 2 cores. If capping per-core
    # queries at a q-tile boundary avoids an extra q-tile for a small
    # overflow, the overflow queries are computed on host instead.
    half_max = max((len(ix) + 1) // 2 for ix in q_idx)
    NQT, QW = _pick_tiling(half_max)
    cap = NQT * QW
    alt = _pick_tiling(max(half_max - MAX_HOST_OVERFLOW // 2, 1))
    if alt[0] * alt[1] < cap and half_max - alt[0] * alt[1] <= MAX_HOST_OVERFLOW // 2:
        NQT, QW = alt
        cap = NQT * QW

    per_core_q = []
    overflow_q = []
    for e in range(N_ELEM):
        ix = q_idx[e]
        half = min((len(ix) + 1) // 2, cap)
        per_core_q.append(ix[:half])
        per_core_q.append(ix[half : 2 * half])
        overflow_q.append(ix[2 * half :])

    max_r = max(max(len(ix) for ix in r_idx), 1)
    NR = math.ceil(max_r / 128)
    R_pad, Q_pad = NR * 128, NQT * QW

    # Per-element (replicated across the 2 cores of a pair) blocked arrays.
    XTs, ATs = [], []
    for e in range(N_ELEM):
        rix = r_idx[e]
        Xe = np.zeros((R_pad, D_FEAT), np.float32)
        Xe[: len(rix)] = X_ref[rix]
        XTd = np.ascontiguousarray(
            Xe.reshape(NR, 128, NF, 128).transpose(0, 3, 2, 1)
        ).reshape(NR, 128, NF * 128)
        XTs.append(XTd)

        Ae = np.zeros((R_pad, P), np.float32)
        Ae[: len(rix)] = Alpha[:, rix].T
        ATd = np.ascontiguousarray(Ae.reshape(NR, 128, P).transpose(1, 0, 2)).reshape(
            128, NR * P
        )
        ATs.append(ATd)

    if USE_BF16:
        import ml_dtypes

        XTs = [x.astype(ml_dtypes.bfloat16) for x in XTs]
        ATs = [a.astype(ml_dtypes.bfloat16) for a in ATs]

    in_maps = []
    for c in range(N_CORES):
        e = c // 2
        qix = per_core_q[c]
        Dq = np.zeros((Q_pad, D_FEAT), np.float32)
        Dq[: len(qix)] = desc[qix]
        DTd = np.ascontiguousarray(
            Dq.reshape(NQT, QW, NF, 128).transpose(0, 3, 2, 1)
        ).reshape(NQT, 128, NF * QW)
        if USE_BF16:
            import ml_dtypes

            DTd = DTd.astype(ml_dtypes.bfloat16)
        in_maps.append({"XT": XTs[e], "DT": DTd, "AT": ATs[e]})

    return in_maps, per_core_q, overflow_q, r_idx, NR, NQT, QW


def kernel(Alpha, X_ref, desc, Z_ref, Z, expK):
    Alpha = np.ascontiguousarray(np.asarray(Alpha, dtype=np.float32))
    X_ref = np.ascontiguousarray(np.asarray(X_ref, dtype=np.float32))
    desc = np.ascontiguousarray(np.asarray(desc, dtype=np.float32))
    Z_ref = np.asarray(Z_ref).astype(np.int64)
    Z = np.asarray(Z).astype(np.int64)
    expK = int(expK)

    if (
        expK != 2
        or Alpha.shape != (P, N_REF)
        or X_ref.shape != (N_REF, D_FEAT)
        or desc.shape != (N_IN, D_FEAT)
        or Z_ref.min() < 0
        or Z_ref.max() >= N_ELEM
        or Z.min() < 0
        or Z.max() >= N_ELEM
    ):
        return _host_fallback(Alpha, X_ref, desc, Z_ref, Z, expK)

    in_maps, per_core_q, overflow_q, r_idx, NR, NQT, QW = _prepare_core_inputs(
        Alpha, X_ref, desc, Z_ref, Z
    )

    from concourse.bass_utils import run_bass_kernel_spmd

    nc = _get_module(NR, NQT, QW)
    import os

    trace = bool(int(os.environ.get("KERNEL_TRACE", "0")))
    res = None
    for attempt in range(2):
        try:
            res = run_bass_kernel_spmd(
                nc,
                in_maps,
                core_ids=list(range(N_CORES)),
                trace=trace,
            )
            break
        except Exception:
            # A previously-profiled session can leave a core transiently
            # unrecoverable; one retry clears it.
            if attempt == 1:
                return _host_fallback(Alpha, X_ref, desc, Z_ref, Z, expK)
    kernel._last_results = res  # for the test harness / profiling

    y = np.zeros((P, N_IN), np.float32)
    for c in range(N_CORES):
        qix = per_core_q[c]
        if len(qix):
            y[:, qix] = res.results[c]["Y"][:, : len(qix)]
    for e in range(N_ELEM):
        oix = overflow_q[e]
        if len(oix):
            rix = r_idx[e]
            K = X_ref[rix] @ desc[oix].T
            y[:, oix] = Alpha[:, rix] @ (K * K)
    return y


# revision 19
# speedup vs baseline: 1.0300x; 1.0300x over previous
"""Trainium2 Bass kernel for the masked per-element grouped kernel problem.

Computes  y = Alpha @ [ (X_ref @ desc.T)**expK  masked to Z_ref[i]==Z[j] ]

Strategy (moe_routing): queries (N_in axis) and reference atoms only interact
when they share an element id (N_ELEM=4). Host-side routing gathers rows by
element; core c handles element e=c//2, query-half h=c%2. Each core runs a
dense pipeline on its element's atoms only:
    K  = X_e @ D_q.T          (contraction 512, bf16 matmuls, fp32 PSUM accum)
    K2 = K**2                 (ScalarE activation Square, PSUM->SBUF, bf16)
    y  = Alpha_e @ K2         (contraction over ref atoms, fp32 PSUM accum)
This is a 4x FLOP reduction over the dense masked formulation, on top of the
8-way data parallelism. A short dummy-matmul warmup flips the PE HAM clock
gate to full rate while the DMA head streams in. A few queries past the last
q-tile boundary (<=64 per element) are computed on host to avoid paying a
whole extra device q-tile for them.
"""

import math

import numpy as np

N_REF, N_IN, D_FEAT, P, N_ELEM = 4096, 8192, 512, 256, 4
N_CORES = 8
NF = D_FEAT // 128  # feature-dim contraction chunks

_COMPILED_CACHE: dict = {}


MAX_HOST_OVERFLOW = 64  # queries/element computed on host when it shaves a q-tile


def _pick_tiling(max_q: int) -> tuple[int, int]:
    """Pick (n_qtiles, qtile_width). Width must be in [256, 512] for full-rate
    matmuls and one PSUM bank; multiple of 32 for clean APs/DMA."""
    max_q = max(max_q, 256)
    n = max(1, math.ceil(max_q / 512))
    qw = math.ceil(max_q / n / 32) * 32
    qw = max(qw, 256)
    return n, qw


def _build_module(NR: int, NQT: int, QW: int, REP: int = 1, use_bf16: bool = False):
    """Build + compile the Bass module for padded shapes.

    Per-core DRAM inputs (blocked layouts produced on host):
      XT [NR, 128, 512]  block r: XT[r, p, f*128+j] = Xe[r*128+j, f*128+p]
      DT [NQT, 128, 4*QW] block qt: DT[qt, p, f*QW+q] = Dq[qt*QW+q, f*128+p]
      AT [128, NR*256]   AT[u, r*256+m] = Ae[r*128+u, m]  (Ae = Alpha[:, rix].T)
    Output:
      Y  [256, NQT*QW]
    """
    import concourse.tile as tile
    from concourse import bacc, mybir

    Q_pad = NQT * QW
    f32 = mybir.dt.float32
    ind = mybir.dt.bfloat16 if use_bf16 else mybir.dt.float32r

    nc = bacc.Bacc(
        "TRN2",
        target_bir_lowering=False,
        debug=False,
        enable_asserts=False,
        num_devices=N_CORES,
    )
    XT = nc.dram_tensor("XT", [NR, 128, NF * 128], ind, kind="ExternalInput")
    DT = nc.dram_tensor("DT", [NQT, 128, NF * QW], ind, kind="ExternalInput")
    AT = nc.dram_tensor("AT", [128, NR * 256], ind, kind="ExternalInput")
    Y = nc.dram_tensor("Y", [P, Q_pad], f32, kind="ExternalOutput")

    with tile.TileContext(nc) as tc:
        with (
            tc.tile_pool(name="xt", bufs=1) as xt_pool,
            tc.tile_pool(name="dt", bufs=1) as dt_pool,
            tc.tile_pool(name="at", bufs=1) as at_pool,
            tc.tile_pool(name="k2", bufs=NR + 2) as k2_pool,
            tc.tile_pool(name="ysb", bufs=1) as y_pool,
            tc.tile_pool(name="kps", bufs=4, space="PSUM") as kps_pool,
            tc.tile_pool(name="yps", bufs=3, space="PSUM") as yps_pool,
        ):
            # Persistent SBUF residents. Distinct tags -> own slots.
            dt_sb = [
                dt_pool.tile([128, NF * QW], ind, tag=f"dt{qt}", name=f"dt_sb{qt}")
                for qt in range(NQT)
            ]
            xt_sb = [
                xt_pool.tile([128, NF * 128], ind, tag=f"xt{r}", name=f"xt_sb{r}")
                for r in range(NR)
            ]
            at_sb = at_pool.tile([128, NR * 256], ind, tag="at", name="at_sb")

            for _rep in range(REP):
                _emit_body(nc, tc, mybir, NR, NQT, QW, XT, DT, AT, Y,
                           dt_sb, xt_sb, at_sb, k2_pool, y_pool, kps_pool, yps_pool,
                           use_bf16)

    nc.compile()
    return nc


def _emit_body(nc, tc, mybir, NR, NQT, QW, XT, DT, AT, Y,
               dt_sb, xt_sb, at_sb, k2_pool, y_pool, kps_pool, yps_pool,
               use_bf16=False):
    f32 = mybir.dt.float32
    ind = mybir.dt.bfloat16 if use_bf16 else mybir.dt.float32r
    SQUARE = mybir.ActivationFunctionType.Square
    P = 256

    # Loads. dt0 + xt* on the sync ring (first q-tile deps first); at and the
    # later q-tiles' desc blocks on the scalar HWDGE ring.
    half = NF * QW // 2
    nc.sync.dma_start(dt_sb[0][:, :half], DT[0][:, :half])
    nc.sync.dma_start(xt_sb[0][:], XT[0])
    nc.sync.dma_start(dt_sb[0][:, half:], DT[0][:, half:])
    for r in range(1, NR):
        nc.sync.dma_start(xt_sb[r][:], XT[r])
    nc.scalar.dma_start(at_sb[:], AT[:])
    for qt in range(1, NQT):
        nc.scalar.dma_start(dt_sb[qt][:], DT[qt])

    y_big = [
        y_pool.tile([128, NQT * QW], f32, tag=f"yb{p}", name=f"y_big{p}")
        for p in range(P // 128)
    ]
    # PE warmup: dummy matmuls against a memset tile (no DMA dependency) keep
    # the PE busy from ~1us through the DMA head, flipping the HAM clock gate
    # to 8/8 and holding it there so every real matmul runs at 2.4 GHz.
    w_ps = kps_pool.tile([128, 512], f32, tag="wps", name="w_ps", bufs=1)
    w_src = y_pool.tile(
        [128, 512], mybir.dt.bfloat16, tag="wsrc", name="w_src", bufs=1
    )
    nc.gpsimd.memset(w_src[:], 0.0)
    for _w in range(15):
        nc.tensor.matmul(
            w_ps[:], w_src[:, 0:128], w_src[:], start=True, stop=True
        )
    for qt in range(NQT):
        k2_tiles = []
        for r in range(NR):
            k_ps = kps_pool.tile([128, QW], f32, tag="kps", name="k_ps")
            for f in range(NF):
                nc.tensor.matmul(
                    k_ps[:],
                    xt_sb[r][:, f * 128 : (f + 1) * 128],
                    dt_sb[qt][:, f * QW : (f + 1) * QW],
                    start=(f == 0),
                    stop=(f == NF - 1),
                )
            k2 = k2_pool.tile([128, QW], ind, tag="k2", name="k2")
            nc.scalar.activation(k2[:], k_ps[:], SQUARE)
            k2_tiles.append(k2)
        for p in range(P // 128):
            y_ps = yps_pool.tile([128, QW], f32, tag="yps", name="y_ps")
            for r in range(NR):
                nc.tensor.matmul(
                    y_ps[:],
                    at_sb[:, r * 256 + p * 128 : r * 256 + (p + 1) * 128],
                    k2_tiles[r][:],
                    start=(r == 0),
                    stop=(r == NR - 1),
                )
            nc.vector.tensor_copy(y_big[p][:, qt * QW : (qt + 1) * QW], y_ps[:])
            nc.sync.dma_start(
                Y[p * 128 : (p + 1) * 128, qt * QW : (qt + 1) * QW],
                y_big[p][:, qt * QW : (qt + 1) * QW],
            )


USE_BF16 = True


def _get_module(NR: int, NQT: int, QW: int):
    key = (NR, NQT, QW, USE_BF16)
    if key not in _COMPILED_CACHE:
        _COMPILED_CACHE[key] = _build_module(NR, NQT, QW, use_bf16=USE_BF16)
    return _COMPILED_CACHE[key]


def _host_fallback(Alpha, X_ref, desc, Z_ref, Z, expK):
    K = np.power(X_ref @ desc.T, expK)
    K *= Z_ref[:, None] == Z[None, :]
    return (Alpha @ K).astype(np.float32)


def _prepare_core_inputs(Alpha, X_ref, desc, Z_ref, Z):
    """Returns (in_maps, per_core_qidx, NR, NQT, QW)."""
    r_idx = [np.nonzero(Z_ref == e)[0] for e in range(N_ELEM)]
    q_idx = [np.nonzero(Z == e)[0] for e in range(N_ELEM)]

    # Split each element's queries across its 2 cores. If capping per-core
    # queries at a q-tile boundary avoids an extra q-tile for a small
    # overflow, the overflow queries are computed on host instead.
    half_max = max((len(ix) + 1) // 2 for ix in q_idx)
    NQT, QW = _pick_tiling(half_max)
    cap = NQT * QW
    alt = _pick_tiling(max(half_max - MAX_HOST_OVERFLOW // 2, 1))
    if alt[0] * alt[1] < cap and half_max - alt[0] * alt[1] <= MAX_HOST_OVERFLOW // 2:
        NQT, QW = alt
        cap = NQT * QW

    per_core_q = []
    overflow_q = []
    for e in range(N_ELEM):
        ix = q_idx[e]
        half = min((len(ix) + 1) // 2, cap)
        per_core_q.append(ix[:half])
        per_core_q.append(ix[half : 2 * half])
        overflow_q.append(ix[2 * half :])

    max_r = max(max(len(ix) for ix in r_idx), 1)
    NR = math.ceil(max_r / 128)
    R_pad, Q_pad = NR * 128, NQT * QW

    # Per-element (replicated across the 2 cores of a pair) blocked arrays.
    XTs, ATs = [], []
    for e in range(N_ELEM):
        rix = r_idx[e]
        Xe = np.zeros((R_pad, D_FEAT), np.float32)
        Xe[: len(rix)] = X_ref[rix]
        XTd = np.ascontiguousarray(
            Xe.reshape(NR, 128, NF, 128).transpose(0, 3, 2, 1)
        ).reshape(NR, 128, NF * 128)
        XTs.append(XTd)

        Ae = np.zeros((R_pad, P), np.float32)
        Ae[: len(rix)] = Alpha[:, rix].T
        ATd = np.ascontiguousarray(Ae.reshape(NR, 128, P).transpose(1, 0, 2)).reshape(
            128, NR * P
        )
        ATs.append(ATd)

    if USE_BF16:
        import ml_dtypes

        XTs = [x.astype(ml_dtypes.bfloat16) for x in XTs]
        ATs = [a.astype(ml_dtypes.bfloat16) for a in ATs]

    in_maps = []
    for c in range(N_CORES):
        e = c // 2
        qix = per_core_q[c]
        Dq = np.zeros((Q_pad, D_FEAT), np.float32)
        Dq[: len(qix)] = desc[qix]
        DTd = np.ascontiguousarray(
            Dq.reshape(NQT, QW, NF, 128).transpose(0, 3, 2, 1)
        ).reshape(NQT, 128, NF * QW)
        if USE_BF16:
            import ml_dtypes

            DTd = DTd.astype(ml_dtypes.bfloat16)
        in_maps.append({"XT": XTs[e], "DT": DTd, "AT": ATs[e]})

    return in_maps, per_core_q, overflow_q, r_idx, NR, NQT, QW


def kernel(Alpha, X_ref, desc, Z_ref, Z, expK):
    Alpha = np.ascontiguousarray(np.asarray(Alpha, dtype=np.float32))
    X_ref = np.ascontiguousarray(np.asarray(X_ref, dtype=np.float32))
    desc = np.ascontiguousarray(np.asarray(desc, dtype=np.float32))
    Z_ref = np.asarray(Z_ref).astype(np.int64)
    Z = np.asarray(Z).astype(np.int64)
    expK = int(expK)

    if (
        expK != 2
        or Alpha.shape != (P, N_REF)
        or X_ref.shape != (N_REF, D_FEAT)
        or desc.shape != (N_IN, D_FEAT)
        or Z_ref.min() < 0
        or Z_ref.max() >= N_ELEM
        or Z.min() < 0
        or Z.max() >= N_ELEM
    ):
        return _host_fallback(Alpha, X_ref, desc, Z_ref, Z, expK)

    in_maps, per_core_q, overflow_q, r_idx, NR, NQT, QW = _prepare_core_inputs(
        Alpha, X_ref, desc, Z_ref, Z
    )

    from concourse.bass_utils import run_bass_kernel_spmd

    nc = _get_module(NR, NQT, QW)
    import os

    trace = bool(int(os.environ.get("KERNEL_TRACE", "0")))
    res = None
    for attempt in range(2):
        try:
            res = run_bass_kernel_spmd(
                nc,
                in_maps,
                core_ids=list(range(N_CORES)),
                trace=trace,
            )
            break
        except Exception:
            # A previously-profiled session can leave a core transiently
            # unrecoverable; one retry clears it.
            if attempt == 1:
                return _host_fallback(Alpha, X_ref, desc, Z_ref, Z, expK)
    kernel._last_results = res  # for the test harness / profiling

    y = np.zeros((P, N_IN), np.float32)
    for c in range(N_CORES):
        qix = per_core_q[c]
        if len(qix):
            y[:, qix] = res.results[c]["Y"][:, : len(qix)]
    for e in range(N_ELEM):
        oix = overflow_q[e]
        if len(oix):
            rix = r_idx[e]
            K = X_ref[rix] @ desc[oix].T
            y[:, oix] = Alpha[:, rix] @ (K * K)
    return y
